# revision 47
# baseline (speedup 1.0000x reference)
"""Trainium2 Bass kernel for one pre-LN transformer block (B=4, T=1024, C=1024,
H=16 heads, FF=4096), distributed over 8 NeuronCores with no collectives.

Sharding: core = (batch b, query-parity j). Each core computes K/V for all 1024
tokens of its batch but attention/FFN only for its 512 queries (tokens t with
t % 2 == j). Interleaved queries make the causal-mask tile structure identical
on every core (SPMD-safe) while skipping ~37.5% of score/AV work. The host
only permutes/transposes inputs and re-interleaves the outputs.

On-device layout: activations live transposed [feature, token] so the whole
chain (LN1 -> QKV -> scores -> AV -> proj -> LN2 -> FFN) is matmul-native.
LayerNorm stats are computed with ones-vector matmuls on the PE and applied
via rank-1 outer-product broadcast matmuls (G0 = g (x) rstd, G1 = g (x)
(-mu*rstd) + b (x) 1).

Precision: matmuls accumulate in f32 PSUM. Scores/AV/FFN operands are bf16
(full-rate, and FWL makes LDWEIGHTS ~2x cheaper than f32r, which dominated
the original PE timeline). The four attention projections (Wq/Wk/Wv/Wproj
and their activations) run in fp8e4m3 DoubleRow (2 contraction rows per
instruction), weights host-prescaled by 64: the 64^2 on q.k cancels in the
softmax exp scale, v/proj unscale at PSUM evacuation. Measured rel err
~1.54e-2 against the f32 reference (gate 2e-2); fp8 on the FFN as well
would land at ~2.3e-2 and fail.
"""

import math
import sys
from dataclasses import dataclass

if "/opt/trn_rl_repo" not in sys.path:
    sys.path.insert(0, "/opt/trn_rl_repo")

import numpy as np


@dataclass(frozen=True)
class Cfg:
    B: int = 4
    T: int = 1024
    C: int = 1024
    H: int = 16
    FF: int = 4096

    @property
    def HD(self):
        return self.C // self.H

    @property
    def TQ(self):  # queries per core
        return self.T // 2

    @property
    def NCI(self):  # C / 128 feature tiles
        return self.C // 128

    @property
    def NFF(self):  # FF / 128 hidden tiles
        return self.FF // 128

    @property
    def NKB(self):  # key blocks of 128
        return self.T // 128

    @property
    def BW(self):  # token block width for LN1 / K phases
        return min(512, self.T)

    @property
    def NTB(self):  # token blocks over all T tokens
        return self.T // self.BW

    @property
    def NQB(self):  # 512-col blocks over TQ (==1 at full size)
        return self.TQ // 512 if self.TQ >= 512 else 1

    @property
    def QW(self):  # query block width
        return min(self.TQ, 512)

    def s_kb(self, kb: int) -> int:
        """Start query-column of the computed score region for key block kb.
        Key blocks 0..NKB/2-1 hold this core's own-parity tokens in order;
        NKB/2.. hold the complementary-parity tokens. Causality allows a
        suffix of queries per block (bf16 matmul is full-rate at any
        moving size, so no >=256 clamp)."""
        half = self.NKB // 2
        kbp = kb % half
        return min(128 * kbp, self.TQ)

    def mask_w(self, kb: int) -> int:
        """Width of the region of score columns that needs the additive mask
        (partially-visible diagonal zone plus any fully-masked overcompute
        from the fp32r clamp)."""
        half = self.NKB // 2
        kbp = kb % half
        end = min(128 * kbp + 128, self.TQ)
        return end - self.s_kb(kb)

    @property
    def MW(self):
        return max(self.mask_w(kb) for kb in range(self.NKB))

    @property
    def pt_offs(self):
        """Column offsets of each key block's packed score region."""
        offs, o = [], 0
        for kb in range(self.NKB):
            offs.append(o)
            o += self.TQ - self.s_kb(kb)
        return offs + [o]


def build_nc(cfg: Cfg, n_cores: int = 8):
    import concourse.bass as bass
    import concourse.tile as tile
    from concourse import bacc, mybir

    f32 = mybir.dt.float32
    f32r = mybir.dt.float32r
    Act = mybir.ActivationFunctionType
    Alu = mybir.AluOpType

    C, H, HD, FF = cfg.C, cfg.H, cfg.HD, cfg.FF
    NCI, NFF, NKB, NTB = cfg.NCI, cfg.NFF, cfg.NKB, cfg.NTB
    TQ, QW, T = cfg.TQ, cfg.QW, cfg.T
    scale = 1.0 / math.sqrt(HD)
    WSCALE = 64.0

    nc = bacc.Bacc(
        "TRN2", target_bir_lowering=False, debug=False, num_devices=n_cores
    )

    # ---- DRAM I/O ----
    bf16 = mybir.dt.bfloat16
    xpt = nc.dram_tensor("xpt", [C, T], bf16, kind="ExternalInput")
    msk = nc.dram_tensor("msk", [NKB, 128, cfg.MW], bf16, kind="ExternalInput")
    ident = nc.dram_tensor("ident", [HD, 256], bf16, kind="ExternalInput")
    dscr_a = nc.dram_tensor("dscr_a", [H * TQ], bf16, kind="Internal")
    dscr_b = nc.dram_tensor("dscr_b", [H * TQ], bf16, kind="Internal")
    # attention projection weights ship pre-scaled by WSCALE in fp8e4m3;
    # QKV/proj matmuls run in DoubleRow mode (2 fp8 weights per PE cell =
    # 2 contraction rows per instruction). The 64x on q,k cancels inside
    # the softmax exp scale; v and proj unscale at PSUM evacuation.
    fp8 = mybir.dt.float8e4
    wq = nc.dram_tensor("wq", [C, C], fp8, kind="ExternalInput")
    wk = nc.dram_tensor("wk", [C, C], fp8, kind="ExternalInput")
    wv = nc.dram_tensor("wv", [C, C], fp8, kind="ExternalInput")
    wp = nc.dram_tensor("wp", [C, C], fp8, kind="ExternalInput")
    w1 = nc.dram_tensor("w1", [C, FF], bf16, kind="ExternalInput")
    w2 = nc.dram_tensor("w2", [FF, C], bf16, kind="ExternalInput")
    # every per-feature param vector packed into one [128, .] tensor: each
    # dma_start costs ~650ns of queue issue time, and 7 serial issues were
    # part of the 18us startup bubble
    NPRM = 6 * NCI + NFF
    prmd = nc.dram_tensor("prm", [128, NPRM], f32, kind="ExternalInput")
    yt = nc.dram_tensor("yt", [C, TQ], f32, kind="ExternalOutput")

    with (
        nc.allow_low_precision(reason="float32r rounding of matmul operands"),
        tile.TileContext(nc) as tc,
    ):
        # ---------------- persistent constants / params ----------------
        # memset cannot target float32r tiles (ISA dtype check); memset an
        # f32 scratch once and cast-copy ones into the matmul constants.
        onesf, free_onesf = tc.tile([128, 128], f32, name="onesf")
        nc.vector.memset(onesf, 1.0)
        ones128, free_ones128 = tc.tile([128, 1], bf16, name="ones128")
        nc.vector.tensor_copy(out=ones128, in_=onesf[:, 0:1])
        ones_row, free_ones_row = tc.tile([1, 128], bf16, name="ones_row")
        nc.vector.tensor_copy(out=ones_row, in_=onesf[0:1, 0:128])
        # lhsT row of ones at partition 64 for the per-head recip broadcast
        oneshi, free_oneshi = tc.tile([65, HD], bf16, name="oneshi")
        nc.vector.tensor_copy(out=oneshi, in_=onesf[0:65, 0:HD])
        epst, free_epst = tc.tile([1, 1], f32, name="epst")
        nc.vector.memset(epst, 1e-5)
        identsb, free_identsb = tc.tile([HD, 256], bf16, name="identsb")

        prm, free_prm = tc.tile([128, NPRM], f32, name="prm")
        nc.scalar.dma_start(out=prm, in_=prmd[:, :])
        ln1gp = prm[:, 0 * NCI : 1 * NCI]
        ln1bp = prm[:, 1 * NCI : 2 * NCI]
        ln2gp = prm[:, 2 * NCI : 3 * NCI]
        ln2bp = prm[:, 3 * NCI : 4 * NCI]
        bpjt = prm[:, 4 * NCI : 5 * NCI]
        b2t = prm[:, 5 * NCI : 6 * NCI]
        b1t = prm[:, 6 * NCI : 6 * NCI + NFF]
        # one PSUM pool + one weight-stream pool for the whole kernel:
        # per-phase pools would serialize phases at their alloc/release
        # boundaries (a pool alloc waits on the previous pool's release,
        # which waits on its last reader).
        ps_all = tc.alloc_tile_pool(name="ps_all", bufs=8, space="PSUM")
        wstream = tc.alloc_tile_pool(name="wstream", bufs=8)

        # x2T = x + attnproj (residual 1), written in the proj phase
        x2t, free_x2t = tc.tile([128, NCI, TQ], bf16, name="x2t")

        mskt, free_mskt = tc.tile([128, NKB, cfg.MW], bf16, name="mskt")

        # raw x^T stays resident through the proj residual add (bf16 makes
        # it cheap); DMA token-block-major so LN1's first stats block can
        # start after half the input has landed.
        raw, free_raw = tc.tile([128, NCI, T], bf16, name="raw")
        xpt_r = xpt.rearrange("(ci p) t -> ci p t", p=128)
        dmaq = (nc.sync, nc.scalar, nc.gpsimd)
        for tb in range(NTB):
            sl = slice(tb * cfg.BW, (tb + 1) * cfg.BW)
            for ci in range(NCI):
                dmaq[(tb * NCI + ci) % 3].dma_start(
                    out=raw[:, ci, sl], in_=xpt_r[ci][:, sl]
                )
        nc.sync.dma_start(out=identsb, in_=ident[:, :])
        nc.sync.dma_start(out=mskt, in_=msk.rearrange("k p m -> p k m"))

        def layernorm(src_ap_fn, dst, gp, bp, n_blocks, blk_w, scopename, psp):
            """src_ap_fn(ci, sl) -> [128, blk_w] f32r AP; dst [128, NCI, *].
            Stats via ones-vector matmuls; per-token scale/shift broadcast via
            two K=1 matmuls per block; gamma/beta applied as ACT Identity
            with per-partition scale/bias. Phases are split across blocks so
            one block's PE stats overlap another block's row math."""
            with (
                nc.named_scope(scopename),
                tc.tile_pool(name=f"{scopename}_sb", bufs=max(3, n_blocks + 1)) as sbp,
            ):
                psp1 = psp
                stats = []
                for tb in range(n_blocks):
                    sl = slice(tb * blk_w, (tb + 1) * blk_w)
                    psx = psp1.tile([1, blk_w], f32, tag="mm", name=f"psx{tb}")
                    psq = psp1.tile([1, blk_w], f32, tag="mm", name=f"psq{tb}")
                    for ci in range(NCI):
                        nc.tensor.matmul(
                            psx, ones128, src_ap_fn(ci, sl),
                            start=(ci == 0), stop=(ci == NCI - 1),
                        )
                    for ci in range(NCI):
                        x_ap = src_ap_fn(ci, sl)
                        sq = sbp.tile([128, blk_w], bf16, tag="sq", name=f"sq{tb}_{ci}")
                        if ci % 2 == 0:
                            nc.scalar.activation(
                                out=sq, in_=x_ap, func=Act.Square
                            )
                        else:
                            nc.vector.tensor_mul(out=sq, in0=x_ap, in1=x_ap)
                        nc.tensor.matmul(
                            psq, ones128, sq,
                            start=(ci == 0), stop=(ci == NCI - 1),
                        )
                    stats.append((psx, psq))
                bcs = []
                mids = []
                for tb in range(n_blocks):
                    psx, psq = stats[tb]
                    # all row scaling/multiplies on DVE: the Scalar engine
                    # then only runs Ln and Exp here, and batching the Lns
                    # before the Exps avoids the 1.3us ACT_TABLE_LOAD
                    # alternation between the natural_log and exp sets
                    mu = sbp.tile([1, blk_w], f32r, tag=f"r0_{tb}", bufs=1)
                    nc.vector.tensor_scalar_mul(out=mu, in0=psx, scalar1=1.0 / C)
                    ms = sbp.tile([1, blk_w], f32r, tag=f"r1_{tb}", bufs=1)
                    nc.vector.tensor_scalar_mul(out=ms, in0=psq, scalar1=1.0 / C)
                    nmu = sbp.tile([1, blk_w], bf16, tag=f"r6_{tb}", bufs=1)
                    nc.vector.tensor_scalar_mul(out=nmu, in0=mu, scalar1=-1.0)
                    mu2 = sbp.tile([1, blk_w], f32r, tag=f"r2_{tb}", bufs=1)
                    nc.vector.tensor_mul(out=mu2, in0=mu, in1=mu)
                    var = sbp.tile([1, blk_w], f32r, tag=f"r3_{tb}", bufs=1)
                    nc.vector.tensor_sub(out=var, in0=ms, in1=mu2)
                    # rstd = exp(-0.5*ln(var+eps)): two fast ACT row ops
                    # instead of sqrt + single-lane DVE reciprocal (~4us)
                    sd = sbp.tile([1, blk_w], f32r, tag=f"r4_{tb}", bufs=1)
                    nc.scalar.activation(
                        out=sd, in_=var, func=Act.Ln, bias=epst
                    )
                    mids.append((sd, nmu))
                for tb in range(n_blocks):
                    sd, nmu = mids[tb]
                    c0 = sbp.tile([1, blk_w], bf16, tag=f"r5_{tb}", bufs=1)
                    nc.scalar.activation(
                        out=c0, in_=sd, func=Act.Exp, scale=-0.5
                    )
                    c1 = sbp.tile([1, blk_w], bf16, tag=f"r7_{tb}", bufs=1)
                    nc.vector.tensor_mul(out=c1, in0=nmu, in1=c0)
                    bc0 = psp.tile([128, blk_w], f32, tag="mm", name=f"bc0_{tb}")
                    bc1 = psp.tile([128, blk_w], f32, tag="mm", name=f"bc1_{tb}")
                    nc.tensor.matmul(bc0, ones_row, c0)
                    nc.tensor.matmul(bc1, ones_row, c1)
                    bcs.append((bc0, bc1))
                for tb in range(n_blocks):
                    sl = slice(tb * blk_w, (tb + 1) * blk_w)
                    bc0, bc1 = bcs[tb]
                    for ci in range(NCI):
                        x_ap = src_ap_fn(ci, sl)
                        tmp = sbp.tile([128, blk_w], f32, tag="tmp", name=f"t{tb}_{ci}")
                        nc.vector.tensor_mul(out=tmp, in0=x_ap, in1=bc0)
                        tmp2 = sbp.tile([128, blk_w], f32, tag="tmp2", name=f"t2_{tb}_{ci}")
                        nc.vector.tensor_add(out=tmp2, in0=tmp, in1=bc1)
                        nc.scalar.activation(
                            out=dst[:, ci, sl], in_=tmp2,
                            func=Act.Identity,
                            bias=bp[:, ci : ci + 1],
                            scale=gp[:, ci : ci + 1],
                        )

        # ---------------- LN1 over all T tokens ----------------
        a1, free_a1 = tc.tile([128, NCI, T], fp8, name="a1", side="right")
        layernorm(lambda ci, sl: raw[:, ci, sl], a1, ln1gp, ln1bp, NTB, cfg.BW, "ln1", ps_all)

        # ---------------- QKV ----------------
        qt, free_qt = tc.tile([128, NCI, TQ], bf16, name="qt")
        kt, free_kt = tc.tile([128, NCI, T], bf16, name="kt")
        vt, free_vt = tc.tile([128, NKB, H, HD + 1], bf16, name="vt")
        for kb in range(NKB):  # ones column for the fused denominator row
            nc.vector.tensor_copy(
                out=vt[:, kb, :, HD : HD + 1], in_=onesf[:, 0:H].unsqueeze(2)
            )

        DR = mybir.MatmulPerfMode.DoubleRow
        NG = NCI // 2
        wqr = wq.rearrange("(g two p) f -> g p two f", two=2, p=128)
        wkr = wk.rearrange("(g two p) f -> g p two f", two=2, p=128)
        wvr = wv.rearrange("(g two p) f -> g p two f", two=2, p=128)
        with nc.named_scope("qkv"):
            wpool = wstream
            psp = ps_all
            # Q: out [C, TQ] (tq blocks of <=512)
            for qb in range(cfg.NQB):
                qsl = slice(qb * QW, (qb + 1) * QW)
                pq = [psp.tile([128, QW], f32, tag="mm", name=f"pq{i}") for i in range(NCI)]
                for g in range(NG):
                    wt = wpool.tile([128, 2, C], fp8, tag="w")
                    nc.sync.dma_start(out=wt, in_=wqr[g])
                    for co in range(NCI):
                        nc.tensor.matmul(
                            pq[co],
                            wt[:, :, 128 * co : 128 * (co + 1)],
                            a1[:, 2 * g : 2 * g + 2, qsl],
                            start=(g == 0), stop=(g == NG - 1),
                            perf_mode=DR,
                        )
                for co in range(NCI):
                    if co % 2 == 0:
                        nc.scalar.copy(out=qt[:, co, qsl], in_=pq[co])
                    else:
                        nc.vector.tensor_copy(out=qt[:, co, qsl], in_=pq[co])
            # K: out [C, T], token blocks
            for tb in range(NTB):
                sl = slice(tb * cfg.BW, (tb + 1) * cfg.BW)
                pk = [psp.tile([128, cfg.BW], f32, tag="mm", name=f"pk{i}") for i in range(NCI)]
                for g in range(NG):
                    wt = wpool.tile([128, 2, C], fp8, tag="w")
                    nc.sync.dma_start(out=wt, in_=wkr[g])
                    for co in range(NCI):
                        nc.tensor.matmul(
                            pk[co],
                            wt[:, :, 128 * co : 128 * (co + 1)],
                            a1[:, 2 * g : 2 * g + 2, sl],
                            start=(g == 0), stop=(g == NG - 1),
                            perf_mode=DR,
                        )
                for co in range(NCI):
                    if co % 2 == 0:
                        nc.scalar.copy(out=kt[:, co, sl], in_=pk[co])
                    else:
                        nc.vector.tensor_copy(out=kt[:, co, sl], in_=pk[co])
            # V: normal layout [tk, d] per key block; lhsT = activations
            vw = min(C, 512)
            nhalf = C // vw  # <=512-wide chunks of the d_all dimension
            hpc = vw // HD  # heads per chunk
            grp = 8 // nhalf
            for kbg in range(math.ceil(NKB / grp)):
                kbs = range(kbg * grp, min(NKB, (kbg + 1) * grp))
                pv = {
                    (kb, hf): psp.tile(
                        [128, vw], f32, tag="mm", name=f"pv{kb}_{hf}"
                    )
                    for kb in kbs
                    for hf in range(nhalf)
                }
                for g in range(NG):
                    wt = wpool.tile([128, 2, C], fp8, tag="w")
                    nc.sync.dma_start(out=wt, in_=wvr[g])
                    for kb in kbs:
                        for hf in range(nhalf):
                            nc.tensor.matmul(
                                pv[kb, hf],
                                a1[:, 2 * g : 2 * g + 2, 128 * kb : 128 * (kb + 1)],
                                wt[:, :, vw * hf : vw * (hf + 1)],
                                start=(g == 0), stop=(g == NG - 1),
                                perf_mode=DR,
                            )
                for kb in kbs:
                    for hf in range(nhalf):
                        # v came out scaled by WSCALE (fp8 weights); divide
                        # back during PSUM evacuation (free on either engine)
                        if (kb + hf) % 2 == 0:
                            nc.vector.tensor_scalar_mul(
                                out=vt[:, kb, hpc * hf : hpc * (hf + 1), 0:HD],
                                in0=pv[kb, hf].rearrange(
                                    "p (h d) -> p h d", h=hpc
                                ),
                                scalar1=1.0 / WSCALE,
                            )
                        else:
                            nc.scalar.mul(
                                out=vt[:, kb, hpc * hf : hpc * (hf + 1), 0:HD],
                                in_=pv[kb, hf].rearrange(
                                    "p (h d) -> p h d", h=hpc
                                ),
                                mul=1.0 / WSCALE,
                            )
        free_a1()
        # QKV's PSUM evacuations are attention's data dependency anyway, so
        # releasing the shared pool here costs nothing and frees all 8 banks
        # for attention-private pools: score tiles (fast release via exp)
        # rotate separately from the long-held AV accumulators, so the
        # scores of pair hp+1 can issue while AV of pair hp still holds its
        # bank -- that hazard is what sank the earlier pipeline attempt.
        ps_all.release()

        # ---------------- attention ----------------
        # att holds, per head, O^T rows 0..HD-1 (unnormalized, then
        # normalized in place) and the softmax denominator (then its
        # reciprocal) in row 64.
        att, free_att = tc.tile([65, H, TQ], bf16, name="att", side="right")
        # packed normalized heads, fp8 for the DoubleRow out-projection
        att2, free_att2 = tc.tile([128, NCI, TQ], fp8, name="att2")
        offs = cfg.pt_offs
        with (
            nc.named_scope("attn"),
            tc.tile_pool(name="ps_sc", bufs=5, space="PSUM") as ps_sc,
            tc.tile_pool(name="ps_av", bufs=2, space="PSUM") as ps_av,
            tc.tile_pool(name="at_pt", bufs=2, side="right") as ptp,
        ):
            pssc = ps_sc
            psav = ps_av

            def scores_block(hp):
                """Emit scores + exp + mask for head pair hp; return pts."""
                heads = (2 * hp, 2 * hp + 1)
                pts = [
                    ptp.tile([128, offs[-1]], bf16, tag="pt", name=f"pt{h}")
                    for h in heads
                ]
                for kb in range(NKB):
                    s = cfg.s_kb(kb)
                    n = TQ - s
                    w = cfg.mask_w(kb)
                    kbsl = slice(128 * kb, 128 * (kb + 1))
                    # interleave the two heads so consecutive matmuls hit
                    # different PE row groups (LDWEIGHTS pulls ahead)
                    pss = []
                    for idx, h in enumerate(heads):
                        po = idx * HD
                        ps_s = pssc.tile([128, 512], f32, tag="mm", name=f"sc{h}")
                        nc.tensor.matmul(
                            ps_s[:, 0:n],
                            kt[po : po + HD, hp, kbsl],
                            qt[po : po + HD, hp, s:TQ],
                        )
                        pss.append(ps_s)
                    for idx, h in enumerate(heads):
                        nc.scalar.activation(
                            out=pts[idx][:, offs[kb] : offs[kb] + n],
                            in_=pss[idx][:, 0:n],
                            func=Act.Exp, scale=scale / (WSCALE * WSCALE),
                        )
                        # causal mask: multiply the diagonal zone by 0/1
                        # (on GpSimd: SBUF-only op, keeps DVE/ACT off the
                        # exp->AV critical chain)
                        nc.gpsimd.tensor_mul(
                            out=pts[idx][:, offs[kb] : offs[kb] + w],
                            in0=pts[idx][:, offs[kb] : offs[kb] + w],
                            in1=mskt[:, kb, 0:w],
                        )
                return pts

            def av_block(hp, pts):
                heads = (2 * hp, 2 * hp + 1)
                for idx, h in enumerate(heads):
                    ps_o = psav.tile([65, TQ], f32, tag="mm", name=f"av{h}")
                    for kb in range(NKB):
                        s = cfg.s_kb(kb)
                        nc.tensor.matmul(
                            ps_o[:, s:TQ],
                            vt[:, kb, h, :],
                            pts[idx][:, offs[kb] : offs[kb + 1]],
                            start=(kb == 0), stop=(kb == NKB - 1),
                            skip_group_check=True,
                        )
                    nc.vector.tensor_copy(
                        out=att[0:65, h, :], in_=ps_o[0:65, :]
                    )
                # after pairs 3 and 7: batch-reciprocal the denominator
                # rows written so far via a DRAM round-trip that spreads
                # them over 128 partitions (overlaps later pairs' matmuls)
                if hp % (H // 4) == H // 4 - 1:
                    half = hp // (H // 4)
                    hsl = slice(half * H // 2, (half + 1) * H // 2)
                    nc.sync.dma_start(
                        out=dscr_a.rearrange("(o h t) -> o h t", o=1, h=H)[
                            :, hsl, :
                        ],
                        in_=att[64:65, hsl, :],
                    )
                    dwide = ptp.tile(
                        [128, H * TQ // 256], bf16, tag="dw", name=f"dw{half}"
                    )
                    nc.sync.dma_start(
                        out=dwide,
                        in_=dscr_a.rearrange(
                            "(bb p f) -> bb p f", bb=2, p=128
                        )[half],
                    )
                    nc.vector.reciprocal(out=dwide, in_=dwide)
                    nc.sync.dma_start(
                        out=dscr_b.rearrange(
                            "(bb p f) -> bb p f", bb=2, p=128
                        )[half],
                        in_=dwide,
                    )
                    nc.sync.dma_start(
                        out=att[64:65, hsl, :],
                        in_=dscr_b.rearrange("(o h t) -> o h t", o=1, h=H)[
                            :, hsl, :
                        ],
                    )

            for hp in range(H // 2):
                av_block(hp, scores_block(hp))
        # attention pools released here; the follow-on pool serves the
        # normalize/pack epilogue and everything after, so proj overlaps it
        ps2 = tc.alloc_tile_pool(name="ps_all2", bufs=8, space="PSUM")
        with nc.named_scope("attn2"):
            for hp in range(H // 2):
                heads = (2 * hp, 2 * hp + 1)
                for qb in range(cfg.NQB):
                    qsl = slice(qb * QW, (qb + 1) * QW)
                    for idx, h in enumerate(heads):
                        bc = ps2.tile([64, QW], f32, tag="mm", name=f"bc{h}")
                        nc.tensor.matmul(
                            bc, oneshi[64:65, :], att[64:65, h, qsl]
                        )
                        nc.vector.tensor_mul(
                            out=att[0:64, h, qsl],
                            in0=att[0:64, h, qsl],
                            in1=bc,
                        )
                    pk = ps2.tile([128, QW], f32, tag="mm", name=f"pk{hp}")
                    nc.tensor.matmul(
                        pk, identsb[:, 0:128], att[0:64, heads[0], qsl],
                        start=True, stop=False,
                    )
                    nc.tensor.matmul(
                        pk, identsb[:, 128:256], att[0:64, heads[1], qsl],
                        start=False, stop=True,
                    )
                    nc.vector.tensor_copy(out=att2[:, hp, qsl], in_=pk)

        # ---------------- attention out-proj + residual 1 ----------------
        wpr = wp.rearrange("(g two p) f -> g p two f", two=2, p=128)
        with nc.named_scope("proj"):
            wpool = wstream
            psp = ps2
            for qb in range(cfg.NQB):
                qsl = slice(qb * QW, (qb + 1) * QW)
                pp = [psp.tile([128, QW], f32, tag="mm", name=f"pp{i}") for i in range(NCI)]
                for g in range(NG):
                    wt = wpool.tile([128, 2, C], fp8, tag="w")
                    nc.sync.dma_start(out=wt, in_=wpr[g])
                    for co in range(NCI):
                        nc.tensor.matmul(
                            pp[co],
                            wt[:, :, 128 * co : 128 * (co + 1)],
                            att2[:, 2 * g : 2 * g + 2, qsl],
                            start=(g == 0), stop=(g == NG - 1),
                            perf_mode=DR,
                        )
                for co in range(NCI):
                    ptmp = wpool.tile([128, QW], f32, tag="pt", name=f"ptm{co}")
                    nc.scalar.activation(
                        out=ptmp,
                        in_=pp[co],
                        func=Act.Identity,
                        bias=bpjt[:, co : co + 1],
                        scale=1.0 / WSCALE,
                    )
                    nc.vector.tensor_add(
                        out=x2t[:, co, qsl], in0=ptmp, in1=raw[:, co, qsl]
                    )
        free_att()
        free_att2()
        free_vt()
        free_kt()
        free_qt()
        free_raw()
        free_mskt()
        yts, free_yts = tc.tile([128, NCI, TQ], f32, name="yts")

        # ---------------- LN2 ----------------
        a2, free_a2 = tc.tile([128, NCI, TQ], bf16, name="a2", side="right")
        layernorm(
            lambda ci, sl: x2t[:, ci, sl], a2, ln2gp, ln2bp, cfg.NQB, QW,
            "ln2", ps2,
        )

        # ---------------- FFN ----------------
        hsb, free_hsb = tc.tile([128, NFF, QW], bf16, name="hsb", side="right")
        with nc.named_scope("ffn1"):
            wpool = wstream
            psp = ps2
            for qb in range(cfg.NQB):
                qsl = slice(qb * QW, (qb + 1) * QW)
                for cog in range(NFF // 8):
                    pf = [psp.tile([128, QW], f32, tag="mm", name=f"pf{i}") for i in range(8)]
                    for ci in range(NCI):
                        wt = wpool.tile([128, 1024], bf16, tag="w")
                        nc.sync.dma_start(
                            out=wt,
                            in_=w1[
                                128 * ci : 128 * (ci + 1),
                                1024 * cog : 1024 * (cog + 1),
                            ],
                        )
                        for co in range(8):
                            nc.tensor.matmul(
                                pf[co],
                                wt[:, 128 * co : 128 * (co + 1)],
                                a2[:, ci, qsl],
                                start=(ci == 0), stop=(ci == NCI - 1),
                            )
                    for co in range(8):
                        hco = cog * 8 + co
                        nc.scalar.activation(
                            out=hsb[:, hco, qsl],
                            in_=pf[co],
                            func=Act.Gelu,
                            bias=b1t[:, hco : hco + 1],
                        )

        with nc.named_scope("ffn2"):
            wpool = wstream
            psp = ps2
            for qb in range(cfg.NQB):
                qsl = slice(qb * QW, (qb + 1) * QW)
                py = [psp.tile([128, QW], f32, tag="mm", name=f"py{i}") for i in range(NCI)]
                for fi in range(NFF):
                    wt = wpool.tile([128, C], bf16, tag="w")
                    nc.sync.dma_start(out=wt, in_=w2[128 * fi : 128 * (fi + 1)])
                    for co in range(NCI):
                        nc.tensor.matmul(
                            py[co],
                            wt[:, 128 * co : 128 * (co + 1)],
                            hsb[:, fi, qsl],
                            start=(fi == 0), stop=(fi == NFF - 1),
                        )
                for co in range(NCI):
                    nc.vector.scalar_tensor_tensor(
                        out=yts[:, co, qsl],
                        in0=py[co],
                        scalar=b2t[:, co : co + 1],
                        in1=x2t[:, co, qsl],
                        op0=Alu.add,
                        op1=Alu.add,
                    )
        # per-co output DMAs so the store overlaps the ffn2 epilogue instead
        # of waiting for the whole yts tile
        ytr = yt.rearrange("(ci p) t -> ci p t", p=128)
        for co in range(NCI):
            nc.sync.dma_start(out=ytr[co], in_=yts[:, co, :])
        free_hsb()
        free_a2()
        free_yts()
        free_x2t()
        wstream.release()
        ps2.release()
        free_prm()
        free_identsb()
        free_epst()
        free_oneshi()
        free_ones_row()
        free_ones128()
        free_onesf()

    nc.compile()
    return nc


def prep_core_inputs(cfg: Cfg, inputs: dict, b: int, j: int) -> dict:
    """Host-side slicing/permutation for core (batch b, parity j)."""
    T, TQ, NKB, MW = cfg.T, cfg.TQ, cfg.NKB, cfg.MW
    import ml_dtypes

    x = np.asarray(inputs["x"])
    perm = np.concatenate([np.arange(j, T, 2), np.arange(1 - j, T, 2)])
    xp = x[b][perm]  # [T, C]
    xpt = np.ascontiguousarray(xp.T).astype(ml_dtypes.bfloat16)

    qtok = perm[:TQ]
    ktok = perm
    mask = np.ones((NKB, 128, MW), dtype=np.float32)
    for kb in range(NKB):
        s = cfg.s_kb(kb)
        w = cfg.mask_w(kb)
        kt = ktok[128 * kb : 128 * (kb + 1)]  # [128]
        qt = qtok[s : s + w]  # [w]
        allowed = qt[None, :] >= kt[:, None]  # [128, w]
        mask[kb, :, :w] = np.where(allowed, 1.0, 0.0)
    return {"xpt": xpt, "msk": mask.astype(ml_dtypes.bfloat16)}


def prep_shared_inputs(cfg: Cfg, inputs: dict) -> dict:
    import ml_dtypes

    C = cfg.C
    f32 = np.float32
    bf16 = ml_dtypes.bfloat16

    def wq2d(w):  # [H, C, HD] -> [C, H*HD], pre-scaled for fp8e4m3
        w = np.asarray(w)
        return np.ascontiguousarray(
            w.transpose(1, 0, 2).reshape(C, C) * 64.0
        ).astype(ml_dtypes.float8_e4m3)

    HD = cfg.HD
    ident = np.zeros((HD, 256), dtype=bf16)
    ident[np.arange(HD), np.arange(HD)] = 1.0
    ident[np.arange(HD), 128 + HD + np.arange(HD)] = 1.0
    return {
        "ident": ident,
        "wq": wq2d(inputs["Wq"]),
        "wk": wq2d(inputs["Wk"]),
        "wv": wq2d(inputs["Wv"]),
        "wp": np.ascontiguousarray(np.asarray(inputs["Wproj"]) * 64.0).astype(
            ml_dtypes.float8_e4m3
        ),
        "w1": np.ascontiguousarray(inputs["W1"]).astype(bf16),
        "w2": np.ascontiguousarray(inputs["W2"]).astype(bf16),
        "prm": np.ascontiguousarray(
            np.concatenate(
                [
                    np.asarray(v, f32).reshape(-1, 128).T
                    for v in (
                        inputs["ln1_g"], inputs["ln1_b"],
                        inputs["ln2_g"], inputs["ln2_b"],
                        inputs["bproj"], inputs["b2"], inputs["b1"],
                    )
                ],
                axis=1,
            )
        ),
    }


def run(
    cfg: Cfg, inputs: dict, n_cores: int = 8, trace: bool = False, reps: int = 1
):
    from concourse.bass_utils import run_bass_kernel_spmd

    nc = build_nc(cfg, n_cores=n_cores)
    shared = prep_shared_inputs(cfg, inputs)
    in_maps = []
    cores = []
    for core in range(n_cores):
        b, j = divmod(core, 2)
        b = b % cfg.B
        in_maps.append({**prep_core_inputs(cfg, inputs, b, j), **shared})
        cores.append((b, j))
    res = None
    times = []
    for _ in range(max(1, reps)):
        r = run_bass_kernel_spmd(
            nc, in_maps, core_ids=list(range(n_cores)), trace=trace
        )
        if r.exec_time_ns is not None:
            times.append(r.exec_time_ns)
        if res is None or r.exec_time_ns is None or (
            res.exec_time_ns is not None and r.exec_time_ns < res.exec_time_ns
        ):
            res = r
    if times:
        print(f"exec times: {sorted(times)}")
    out = np.zeros((cfg.B, cfg.T, cfg.C), dtype=np.float32)
    for core, (b, j) in enumerate(cores):
        ytv = res.results[core]["yt"]  # [C, TQ]
        perm = np.concatenate(
            [np.arange(j, cfg.T, 2), np.arange(1 - j, cfg.T, 2)]
        )
        out[b, perm[: cfg.TQ], :] = ytv.T
    return out, res


def kernel(**inputs) -> np.ndarray:
    out, _ = run(Cfg(), inputs, n_cores=8, trace=False)
    return out


if __name__ == "__main__":
    # quick self-exercise at full size with random data
    rng = np.random.default_rng(0)
    cfg = Cfg()
    ins = {
        "x": rng.standard_normal((cfg.B, cfg.T, cfg.C)).astype(np.float32),
        "ln1_g": np.ones(cfg.C, np.float32),
        "ln1_b": np.zeros(cfg.C, np.float32),
        "ln2_g": np.ones(cfg.C, np.float32),
        "ln2_b": np.zeros(cfg.C, np.float32),
        "Wq": rng.standard_normal((cfg.H, cfg.C, cfg.HD)).astype(np.float32)
        * 0.02,
        "Wk": rng.standard_normal((cfg.H, cfg.C, cfg.HD)).astype(np.float32)
        * 0.02,
        "Wv": rng.standard_normal((cfg.H, cfg.C, cfg.HD)).astype(np.float32)
        * 0.02,
        "Wproj": rng.standard_normal((cfg.C, cfg.C)).astype(np.float32) * 0.02,
        "bproj": np.zeros(cfg.C, np.float32),
        "W1": rng.standard_normal((cfg.C, cfg.FF)).astype(np.float32) * 0.02,
        "b1": np.zeros(cfg.FF, np.float32),
        "W2": rng.standard_normal((cfg.FF, cfg.C)).astype(np.float32) * 0.02,
        "b2": np.zeros(cfg.C, np.float32),
    }
    y = kernel(**ins)
    print("ran, out", y.shape, y.dtype, float(np.abs(y).max()))



# revision 48
# speedup vs baseline: 1.0649x; 1.0649x over previous
"""Trainium2 Bass kernel for one pre-LN transformer block (B=4, T=1024, C=1024,
H=16 heads, FF=4096), distributed over 8 NeuronCores with no collectives.

Sharding: core = (batch b, query-parity j). Each core computes K/V for all 1024
tokens of its batch but attention/FFN only for its 512 queries (tokens t with
t % 2 == j). Interleaved queries make the causal-mask tile structure identical
on every core (SPMD-safe) while skipping ~37.5% of score/AV work. The host
only permutes/transposes inputs and re-interleaves the outputs.

On-device layout: activations live transposed [feature, token] so the whole
chain (LN1 -> QKV -> scores -> AV -> proj -> LN2 -> FFN) is matmul-native.
LayerNorm stats are computed with ones-vector matmuls on the PE and applied
via rank-1 outer-product broadcast matmuls (G0 = g (x) rstd, G1 = g (x)
(-mu*rstd) + b (x) 1).

Precision: matmuls accumulate in f32 PSUM. Scores/AV/FFN operands are bf16
(full-rate, and FWL makes LDWEIGHTS ~2x cheaper than f32r, which dominated
the original PE timeline). The four attention projections (Wq/Wk/Wv/Wproj
and their activations) run in fp8e4m3 DoubleRow (2 contraction rows per
instruction), weights host-prescaled by 64: the 64^2 on q.k cancels in the
softmax exp scale, v/proj unscale at PSUM evacuation. Measured rel err
~1.54e-2 against the f32 reference (gate 2e-2); fp8 on the FFN as well
would land at ~2.3e-2 and fail.
"""

import math
import sys
from dataclasses import dataclass

if "/opt/trn_rl_repo" not in sys.path:
    sys.path.insert(0, "/opt/trn_rl_repo")

import numpy as np


@dataclass(frozen=True)
class Cfg:
    B: int = 4
    T: int = 1024
    C: int = 1024
    H: int = 16
    FF: int = 4096

    @property
    def HD(self):
        return self.C // self.H

    @property
    def TQ(self):  # queries per core
        return self.T // 2

    @property
    def NCI(self):  # C / 128 feature tiles
        return self.C // 128

    @property
    def NFF(self):  # FF / 128 hidden tiles
        return self.FF // 128

    @property
    def NKB(self):  # key blocks of 128
        return self.T // 128

    @property
    def BW(self):  # token block width for LN1 / K phases
        return min(512, self.T)

    @property
    def NTB(self):  # token blocks over all T tokens
        return self.T // self.BW

    @property
    def NQB(self):  # 512-col blocks over TQ (==1 at full size)
        return self.TQ // 512 if self.TQ >= 512 else 1

    @property
    def QW(self):  # query block width
        return min(self.TQ, 512)

    def s_kb(self, kb: int) -> int:
        """Start query-column of the computed score region for key block kb.
        Key blocks 0..NKB/2-1 hold this core's own-parity tokens in order;
        NKB/2.. hold the complementary-parity tokens. Causality allows a
        suffix of queries per block (bf16 matmul is full-rate at any
        moving size, so no >=256 clamp)."""
        half = self.NKB // 2
        kbp = kb % half
        return min(128 * kbp, self.TQ)

    def mask_w(self, kb: int) -> int:
        """Width of the region of score columns that needs the additive mask
        (partially-visible diagonal zone plus any fully-masked overcompute
        from the fp32r clamp)."""
        half = self.NKB // 2
        kbp = kb % half
        end = min(128 * kbp + 128, self.TQ)
        return end - self.s_kb(kb)

    @property
    def MW(self):
        return max(self.mask_w(kb) for kb in range(self.NKB))

    @property
    def pt_offs(self):
        """Column offsets of each key block's packed score region."""
        offs, o = [], 0
        for kb in range(self.NKB):
            offs.append(o)
            o += self.TQ - self.s_kb(kb)
        return offs + [o]


def build_nc(cfg: Cfg, n_cores: int = 8):
    import concourse.bass as bass
    import concourse.tile as tile
    from concourse import bacc, mybir

    f32 = mybir.dt.float32
    f32r = mybir.dt.float32r
    Act = mybir.ActivationFunctionType
    Alu = mybir.AluOpType

    C, H, HD, FF = cfg.C, cfg.H, cfg.HD, cfg.FF
    NCI, NFF, NKB, NTB = cfg.NCI, cfg.NFF, cfg.NKB, cfg.NTB
    TQ, QW, T = cfg.TQ, cfg.QW, cfg.T
    scale = 1.0 / math.sqrt(HD)
    WSCALE = 64.0

    nc = bacc.Bacc(
        "TRN2", target_bir_lowering=False, debug=False, num_devices=n_cores
    )

    # ---- DRAM I/O ----
    bf16 = mybir.dt.bfloat16
    xpt = nc.dram_tensor("xpt", [C, T], bf16, kind="ExternalInput")
    msk = nc.dram_tensor("msk", [NKB, 128, cfg.MW], bf16, kind="ExternalInput")
    ident = nc.dram_tensor("ident", [HD, 256], bf16, kind="ExternalInput")
    dscr_a = nc.dram_tensor("dscr_a", [H * TQ], bf16, kind="Internal")
    dscr_b = nc.dram_tensor("dscr_b", [H * TQ], bf16, kind="Internal")
    # attention projection weights ship pre-scaled by WSCALE in fp8e4m3;
    # QKV/proj matmuls run in DoubleRow mode (2 fp8 weights per PE cell =
    # 2 contraction rows per instruction). The 64x on q,k cancels inside
    # the softmax exp scale; v and proj unscale at PSUM evacuation.
    fp8 = mybir.dt.float8e4
    wq = nc.dram_tensor("wq", [C, C], fp8, kind="ExternalInput")
    wk = nc.dram_tensor("wk", [C, C], fp8, kind="ExternalInput")
    wv = nc.dram_tensor("wv", [C, C], fp8, kind="ExternalInput")
    wp = nc.dram_tensor("wp", [C, C], fp8, kind="ExternalInput")
    w1 = nc.dram_tensor("w1", [C, FF], bf16, kind="ExternalInput")
    w2 = nc.dram_tensor("w2", [FF, C], bf16, kind="ExternalInput")
    # every per-feature param vector packed into one [128, .] tensor: each
    # dma_start costs ~650ns of queue issue time, and 7 serial issues were
    # part of the 18us startup bubble
    NPRM = 6 * NCI + NFF
    prmd = nc.dram_tensor("prm", [128, NPRM], f32, kind="ExternalInput")
    yt = nc.dram_tensor("yt", [C, TQ], f32, kind="ExternalOutput")

    with (
        nc.allow_low_precision(reason="float32r rounding of matmul operands"),
        tile.TileContext(nc) as tc,
    ):
        # ---------------- persistent constants / params ----------------
        # memset cannot target float32r tiles (ISA dtype check); memset an
        # f32 scratch once and cast-copy ones into the matmul constants.
        onesf, free_onesf = tc.tile([128, 128], f32, name="onesf")
        nc.vector.memset(onesf, 1.0)
        ones128, free_ones128 = tc.tile([128, 1], bf16, name="ones128")
        nc.vector.tensor_copy(out=ones128, in_=onesf[:, 0:1])
        ones_row, free_ones_row = tc.tile([1, 128], bf16, name="ones_row")
        nc.vector.tensor_copy(out=ones_row, in_=onesf[0:1, 0:128])
        # lhsT row of ones at partition 64 for the per-head recip broadcast
        oneshi, free_oneshi = tc.tile([65, HD], bf16, name="oneshi")
        nc.vector.tensor_copy(out=oneshi, in_=onesf[0:65, 0:HD])
        epst, free_epst = tc.tile([1, 1], f32, name="epst")
        nc.vector.memset(epst, 1e-5)
        identsb, free_identsb = tc.tile([HD, 256], bf16, name="identsb")

        prm, free_prm = tc.tile([128, NPRM], f32, name="prm")
        nc.scalar.dma_start(out=prm, in_=prmd[:, :])
        ln1gp = prm[:, 0 * NCI : 1 * NCI]
        ln1bp = prm[:, 1 * NCI : 2 * NCI]
        ln2gp = prm[:, 2 * NCI : 3 * NCI]
        ln2bp = prm[:, 3 * NCI : 4 * NCI]
        bpjt = prm[:, 4 * NCI : 5 * NCI]
        b2t = prm[:, 5 * NCI : 6 * NCI]
        b1t = prm[:, 6 * NCI : 6 * NCI + NFF]
        # one PSUM pool + one weight-stream pool for the whole kernel:
        # per-phase pools would serialize phases at their alloc/release
        # boundaries (a pool alloc waits on the previous pool's release,
        # which waits on its last reader).
        ps_all = tc.alloc_tile_pool(name="ps_all", bufs=8, space="PSUM")
        wstream = tc.alloc_tile_pool(name="wstream", bufs=8)

        # x2T = x + attnproj (residual 1), written in the proj phase
        x2t, free_x2t = tc.tile([128, NCI, TQ], bf16, name="x2t")

        mskt, free_mskt = tc.tile([128, NKB, cfg.MW], bf16, name="mskt")

        # raw x^T stays resident through the proj residual add (bf16 makes
        # it cheap); DMA token-block-major so LN1's first stats block can
        # start after half the input has landed.
        raw, free_raw = tc.tile([128, NCI, T], bf16, name="raw")
        xpt_r = xpt.rearrange("(ci p) t -> ci p t", p=128)
        dmaq = (nc.sync, nc.scalar, nc.gpsimd)
        for tb in range(NTB):
            sl = slice(tb * cfg.BW, (tb + 1) * cfg.BW)
            for ci in range(NCI):
                dmaq[(tb * NCI + ci) % 3].dma_start(
                    out=raw[:, ci, sl], in_=xpt_r[ci][:, sl]
                )
        nc.sync.dma_start(out=identsb, in_=ident[:, :])
        nc.sync.dma_start(out=mskt, in_=msk.rearrange("k p m -> p k m"))

        def layernorm(src_ap_fn, dst, gp, bp, n_blocks, blk_w, scopename):
            """src_ap_fn(ci, sl) -> [128, blk_w] f32r AP; dst [128, NCI, *].
            Stats via ones-vector matmuls; per-token scale/shift broadcast via
            two K=1 matmuls per block; gamma/beta applied as ACT Identity
            with per-partition scale/bias. Phases are split across blocks so
            one block's PE stats overlap another block's row math."""
            with (
                nc.named_scope(scopename),
                tc.tile_pool(name=f"{scopename}_sb", bufs=max(3, n_blocks + 1)) as sbp,
            ):
                psp = psp1 = ps_all
                stats = []
                for tb in range(n_blocks):
                    sl = slice(tb * blk_w, (tb + 1) * blk_w)
                    psx = psp1.tile([1, blk_w], f32, tag="mm", name=f"psx{tb}")
                    psq = psp1.tile([1, blk_w], f32, tag="mm", name=f"psq{tb}")
                    for ci in range(NCI):
                        nc.tensor.matmul(
                            psx, ones128, src_ap_fn(ci, sl),
                            start=(ci == 0), stop=(ci == NCI - 1),
                        )
                    for ci in range(NCI):
                        x_ap = src_ap_fn(ci, sl)
                        sq = sbp.tile([128, blk_w], bf16, tag="sq", name=f"sq{tb}_{ci}")
                        if ci % 2 == 0:
                            nc.scalar.activation(
                                out=sq, in_=x_ap, func=Act.Square
                            )
                        else:
                            nc.vector.tensor_mul(out=sq, in0=x_ap, in1=x_ap)
                        nc.tensor.matmul(
                            psq, ones128, sq,
                            start=(ci == 0), stop=(ci == NCI - 1),
                        )
                    stats.append((psx, psq))
                bcs = []
                mids = []
                for tb in range(n_blocks):
                    psx, psq = stats[tb]
                    # all row scaling/multiplies on DVE: the Scalar engine
                    # then only runs Ln and Exp here, and batching the Lns
                    # before the Exps avoids the 1.3us ACT_TABLE_LOAD
                    # alternation between the natural_log and exp sets
                    mu = sbp.tile([1, blk_w], f32r, tag=f"r0_{tb}", bufs=1)
                    nc.vector.tensor_scalar_mul(out=mu, in0=psx, scalar1=1.0 / C)
                    ms = sbp.tile([1, blk_w], f32r, tag=f"r1_{tb}", bufs=1)
                    nc.vector.tensor_scalar_mul(out=ms, in0=psq, scalar1=1.0 / C)
                    nmu = sbp.tile([1, blk_w], bf16, tag=f"r6_{tb}", bufs=1)
                    nc.vector.tensor_scalar_mul(out=nmu, in0=mu, scalar1=-1.0)
                    mu2 = sbp.tile([1, blk_w], f32r, tag=f"r2_{tb}", bufs=1)
                    nc.vector.tensor_mul(out=mu2, in0=mu, in1=mu)
                    var = sbp.tile([1, blk_w], f32r, tag=f"r3_{tb}", bufs=1)
                    nc.vector.tensor_sub(out=var, in0=ms, in1=mu2)
                    # rstd = exp(-0.5*ln(var+eps)): two fast ACT row ops
                    # instead of sqrt + single-lane DVE reciprocal (~4us)
                    sd = sbp.tile([1, blk_w], f32r, tag=f"r4_{tb}", bufs=1)
                    nc.scalar.activation(
                        out=sd, in_=var, func=Act.Ln, bias=epst
                    )
                    mids.append((sd, nmu))
                for tb in range(n_blocks):
                    sd, nmu = mids[tb]
                    c0 = sbp.tile([1, blk_w], bf16, tag=f"r5_{tb}", bufs=1)
                    nc.scalar.activation(
                        out=c0, in_=sd, func=Act.Exp, scale=-0.5
                    )
                    c1 = sbp.tile([1, blk_w], bf16, tag=f"r7_{tb}", bufs=1)
                    nc.vector.tensor_mul(out=c1, in0=nmu, in1=c0)
                    bc0 = psp.tile([128, blk_w], f32, tag="mm", name=f"bc0_{tb}")
                    bc1 = psp.tile([128, blk_w], f32, tag="mm", name=f"bc1_{tb}")
                    nc.tensor.matmul(bc0, ones_row, c0)
                    nc.tensor.matmul(bc1, ones_row, c1)
                    bcs.append((bc0, bc1))
                for tb in range(n_blocks):
                    sl = slice(tb * blk_w, (tb + 1) * blk_w)
                    bc0, bc1 = bcs[tb]
                    for ci in range(NCI):
                        x_ap = src_ap_fn(ci, sl)
                        tmp = sbp.tile([128, blk_w], f32, tag="tmp", name=f"t{tb}_{ci}")
                        nc.vector.tensor_mul(out=tmp, in0=x_ap, in1=bc0)
                        tmp2 = sbp.tile([128, blk_w], f32, tag="tmp2", name=f"t2_{tb}_{ci}")
                        nc.vector.tensor_add(out=tmp2, in0=tmp, in1=bc1)
                        nc.scalar.activation(
                            out=dst[:, ci, sl], in_=tmp2,
                            func=Act.Identity,
                            bias=bp[:, ci : ci + 1],
                            scale=gp[:, ci : ci + 1],
                        )

        # ---------------- LN1 over all T tokens ----------------
        a1, free_a1 = tc.tile([128, NCI, T], fp8, name="a1", side="right")
        layernorm(lambda ci, sl: raw[:, ci, sl], a1, ln1gp, ln1bp, NTB, cfg.BW, "ln1")

        # ---------------- QKV ----------------
        qt, free_qt = tc.tile([128, NCI, TQ], bf16, name="qt")
        kt, free_kt = tc.tile([128, NCI, T], bf16, name="kt")
        vt, free_vt = tc.tile([128, NKB, H, HD + 1], bf16, name="vt")
        for kb in range(NKB):  # ones column for the fused denominator row
            nc.vector.tensor_copy(
                out=vt[:, kb, :, HD : HD + 1], in_=onesf[:, 0:H].unsqueeze(2)
            )

        DR = mybir.MatmulPerfMode.DoubleRow
        NG = NCI // 2
        wqr = wq.rearrange("(g two p) f -> g p two f", two=2, p=128)
        wkr = wk.rearrange("(g two p) f -> g p two f", two=2, p=128)
        wvr = wv.rearrange("(g two p) f -> g p two f", two=2, p=128)
        with nc.named_scope("qkv"):
            wpool = wstream
            psp = ps_all
            # Q: out [C, TQ] (tq blocks of <=512)
            for qb in range(cfg.NQB):
                qsl = slice(qb * QW, (qb + 1) * QW)
                pq = [psp.tile([128, QW], f32, tag="mm", name=f"pq{i}") for i in range(NCI)]
                for g in range(NG):
                    wt = wpool.tile([128, 2, C], fp8, tag="w")
                    nc.sync.dma_start(out=wt, in_=wqr[g])
                    for co in range(NCI):
                        nc.tensor.matmul(
                            pq[co],
                            wt[:, :, 128 * co : 128 * (co + 1)],
                            a1[:, 2 * g : 2 * g + 2, qsl],
                            start=(g == 0), stop=(g == NG - 1),
                            perf_mode=DR,
                        )
                for co in range(NCI):
                    if co % 2 == 0:
                        nc.scalar.copy(out=qt[:, co, qsl], in_=pq[co])
                    else:
                        nc.vector.tensor_copy(out=qt[:, co, qsl], in_=pq[co])
            # K: out [C, T], token blocks
            for tb in range(NTB):
                sl = slice(tb * cfg.BW, (tb + 1) * cfg.BW)
                pk = [psp.tile([128, cfg.BW], f32, tag="mm", name=f"pk{i}") for i in range(NCI)]
                for g in range(NG):
                    wt = wpool.tile([128, 2, C], fp8, tag="w")
                    nc.sync.dma_start(out=wt, in_=wkr[g])
                    for co in range(NCI):
                        nc.tensor.matmul(
                            pk[co],
                            wt[:, :, 128 * co : 128 * (co + 1)],
                            a1[:, 2 * g : 2 * g + 2, sl],
                            start=(g == 0), stop=(g == NG - 1),
                            perf_mode=DR,
                        )
                for co in range(NCI):
                    if co % 2 == 0:
                        nc.scalar.copy(out=kt[:, co, sl], in_=pk[co])
                    else:
                        nc.vector.tensor_copy(out=kt[:, co, sl], in_=pk[co])
            # V: normal layout [tk, d] per key block; lhsT = activations
            vw = min(C, 512)
            nhalf = C // vw  # <=512-wide chunks of the d_all dimension
            hpc = vw // HD  # heads per chunk
            grp = 8 // nhalf
            for kbg in range(math.ceil(NKB / grp)):
                kbs = range(kbg * grp, min(NKB, (kbg + 1) * grp))
                pv = {
                    (kb, hf): psp.tile(
                        [128, vw], f32, tag="mm", name=f"pv{kb}_{hf}"
                    )
                    for kb in kbs
                    for hf in range(nhalf)
                }
                for g in range(NG):
                    wt = wpool.tile([128, 2, C], fp8, tag="w")
                    nc.sync.dma_start(out=wt, in_=wvr[g])
                    for kb in kbs:
                        for hf in range(nhalf):
                            nc.tensor.matmul(
                                pv[kb, hf],
                                a1[:, 2 * g : 2 * g + 2, 128 * kb : 128 * (kb + 1)],
                                wt[:, :, vw * hf : vw * (hf + 1)],
                                start=(g == 0), stop=(g == NG - 1),
                                perf_mode=DR,
                            )
                for kb in kbs:
                    for hf in range(nhalf):
                        # v came out scaled by WSCALE (fp8 weights); divide
                        # back during PSUM evacuation (free on either engine)
                        if (kb + hf) % 2 == 0:
                            nc.vector.tensor_scalar_mul(
                                out=vt[:, kb, hpc * hf : hpc * (hf + 1), 0:HD],
                                in0=pv[kb, hf].rearrange(
                                    "p (h d) -> p h d", h=hpc
                                ),
                                scalar1=1.0 / WSCALE,
                            )
                        else:
                            nc.scalar.mul(
                                out=vt[:, kb, hpc * hf : hpc * (hf + 1), 0:HD],
                                in_=pv[kb, hf].rearrange(
                                    "p (h d) -> p h d", h=hpc
                                ),
                                mul=1.0 / WSCALE,
                            )
        free_a1()

        # ---------------- attention ----------------
        # att holds, per head, O^T rows 0..HD-1 (unnormalized, then
        # normalized in place) and the softmax denominator (then its
        # reciprocal) in row 64.
        att, free_att = tc.tile([65, H, TQ], bf16, name="att", side="right")
        # packed normalized heads, fp8 for the DoubleRow out-projection
        att2, free_att2 = tc.tile([128, NCI, TQ], fp8, name="att2")
        offs = cfg.pt_offs
        with (
            nc.named_scope("attn"),
            tc.tile_pool(name="at_pt", bufs=2, side="right") as ptp,
        ):
            pssc = psav = psbc = ps_all

            def scores_block(hp):
                """Emit scores + exp + mask for head pair hp; return pts."""
                heads = (2 * hp, 2 * hp + 1)
                pts = [
                    ptp.tile([128, offs[-1]], bf16, tag="pt", name=f"pt{h}")
                    for h in heads
                ]
                for kb in range(NKB):
                    s = cfg.s_kb(kb)
                    n = TQ - s
                    w = cfg.mask_w(kb)
                    kbsl = slice(128 * kb, 128 * (kb + 1))
                    # interleave the two heads so consecutive matmuls hit
                    # different PE row groups (LDWEIGHTS pulls ahead)
                    pss = []
                    for idx, h in enumerate(heads):
                        po = idx * HD
                        ps_s = pssc.tile([128, 512], f32, tag="mm", name=f"sc{h}")
                        nc.tensor.matmul(
                            ps_s[:, 0:n],
                            kt[po : po + HD, hp, kbsl],
                            qt[po : po + HD, hp, s:TQ],
                        )
                        pss.append(ps_s)
                    for idx, h in enumerate(heads):
                        nc.scalar.activation(
                            out=pts[idx][:, offs[kb] : offs[kb] + n],
                            in_=pss[idx][:, 0:n],
                            func=Act.Exp, scale=scale / (WSCALE * WSCALE),
                        )
                        # causal mask: multiply the diagonal zone by 0/1
                        # (on GpSimd: SBUF-only op, keeps DVE/ACT off the
                        # exp->AV critical chain)
                        nc.gpsimd.tensor_mul(
                            out=pts[idx][:, offs[kb] : offs[kb] + w],
                            in0=pts[idx][:, offs[kb] : offs[kb] + w],
                            in1=mskt[:, kb, 0:w],
                        )
                return pts

            def av_block(hp, pts):
                heads = (2 * hp, 2 * hp + 1)
                for idx, h in enumerate(heads):
                    ps_o = psav.tile([65, TQ], f32, tag="mm", name=f"av{h}")
                    for kb in range(NKB):
                        s = cfg.s_kb(kb)
                        nc.tensor.matmul(
                            ps_o[:, s:TQ],
                            vt[:, kb, h, :],
                            pts[idx][:, offs[kb] : offs[kb + 1]],
                            start=(kb == 0), stop=(kb == NKB - 1),
                            skip_group_check=True,
                        )
                    nc.vector.tensor_copy(
                        out=att[0:65, h, :], in_=ps_o[0:65, :]
                    )
                # after pairs 3 and 7: batch-reciprocal the denominator
                # rows written so far via a DRAM round-trip that spreads
                # them over 128 partitions (overlaps later pairs' matmuls)
                if hp % (H // 4) == H // 4 - 1:
                    half = hp // (H // 4)
                    hsl = slice(half * H // 2, (half + 1) * H // 2)
                    nc.sync.dma_start(
                        out=dscr_a.rearrange("(o h t) -> o h t", o=1, h=H)[
                            :, hsl, :
                        ],
                        in_=att[64:65, hsl, :],
                    )
                    dwide = ptp.tile(
                        [128, H * TQ // 256], bf16, tag="dw", name=f"dw{half}"
                    )
                    nc.sync.dma_start(
                        out=dwide,
                        in_=dscr_a.rearrange(
                            "(bb p f) -> bb p f", bb=2, p=128
                        )[half],
                    )
                    nc.vector.reciprocal(out=dwide, in_=dwide)
                    nc.sync.dma_start(
                        out=dscr_b.rearrange(
                            "(bb p f) -> bb p f", bb=2, p=128
                        )[half],
                        in_=dwide,
                    )
                    nc.sync.dma_start(
                        out=att[64:65, hsl, :],
                        in_=dscr_b.rearrange("(o h t) -> o h t", o=1, h=H)[
                            :, hsl, :
                        ],
                    )

            for hp in range(H // 2):
                av_block(hp, scores_block(hp))
            # normalize all heads and pack pairs to 128 partitions; emitted
            # after the head loop so the bc/pack psum tiles sit at the end
            # of the shared-pool slot rotation (a mid-loop slow tile would
            # stall allocations eight slots later)
            for hp in range(H // 2):
                heads = (2 * hp, 2 * hp + 1)
                for qb in range(cfg.NQB):
                    qsl = slice(qb * QW, (qb + 1) * QW)
                    for idx, h in enumerate(heads):
                        bc = psbc.tile([64, QW], f32, tag="mm", name=f"bc{h}")
                        nc.tensor.matmul(
                            bc, oneshi[64:65, :], att[64:65, h, qsl]
                        )
                        nc.vector.tensor_mul(
                            out=att[0:64, h, qsl],
                            in0=att[0:64, h, qsl],
                            in1=bc,
                        )
                    pk = psbc.tile([128, QW], f32, tag="mm", name=f"pk{hp}")
                    nc.tensor.matmul(
                        pk, identsb[:, 0:128], att[0:64, heads[0], qsl],
                        start=True, stop=False,
                    )
                    nc.tensor.matmul(
                        pk, identsb[:, 128:256], att[0:64, heads[1], qsl],
                        start=False, stop=True,
                    )
                    nc.vector.tensor_copy(out=att2[:, hp, qsl], in_=pk)

        # ---------------- attention out-proj + residual 1 ----------------
        wpr = wp.rearrange("(g two p) f -> g p two f", two=2, p=128)
        with nc.named_scope("proj"):
            wpool = wstream
            psp = ps_all
            for qb in range(cfg.NQB):
                qsl = slice(qb * QW, (qb + 1) * QW)
                pp = [psp.tile([128, QW], f32, tag="mm", name=f"pp{i}") for i in range(NCI)]
                for g in range(NG):
                    wt = wpool.tile([128, 2, C], fp8, tag="w")
                    nc.sync.dma_start(out=wt, in_=wpr[g])
                    for co in range(NCI):
                        nc.tensor.matmul(
                            pp[co],
                            wt[:, :, 128 * co : 128 * (co + 1)],
                            att2[:, 2 * g : 2 * g + 2, qsl],
                            start=(g == 0), stop=(g == NG - 1),
                            perf_mode=DR,
                        )
                for co in range(NCI):
                    ptmp = wpool.tile([128, QW], f32, tag="pt", name=f"ptm{co}")
                    nc.scalar.activation(
                        out=ptmp,
                        in_=pp[co],
                        func=Act.Identity,
                        bias=bpjt[:, co : co + 1],
                        scale=1.0 / WSCALE,
                    )
                    nc.vector.tensor_add(
                        out=x2t[:, co, qsl], in0=ptmp, in1=raw[:, co, qsl]
                    )
        free_att()
        free_att2()
        free_vt()
        free_kt()
        free_qt()
        free_raw()
        free_mskt()
        yts, free_yts = tc.tile([128, NCI, TQ], f32, name="yts")

        # ---------------- LN2 ----------------
        a2, free_a2 = tc.tile([128, NCI, TQ], bf16, name="a2", side="right")
        layernorm(
            lambda ci, sl: x2t[:, ci, sl], a2, ln2gp, ln2bp, cfg.NQB, QW, "ln2"
        )

        # ---------------- FFN ----------------
        hsb, free_hsb = tc.tile([128, NFF, QW], bf16, name="hsb", side="right")
        with nc.named_scope("ffn1"):
            wpool = wstream
            psp = ps_all
            for qb in range(cfg.NQB):
                qsl = slice(qb * QW, (qb + 1) * QW)
                for cog in range(NFF // 8):
                    pf = [psp.tile([128, QW], f32, tag="mm", name=f"pf{i}") for i in range(8)]
                    for ci in range(NCI):
                        wt = wpool.tile([128, 1024], bf16, tag="w")
                        nc.sync.dma_start(
                            out=wt,
                            in_=w1[
                                128 * ci : 128 * (ci + 1),
                                1024 * cog : 1024 * (cog + 1),
                            ],
                        )
                        for co in range(8):
                            nc.tensor.matmul(
                                pf[co],
                                wt[:, 128 * co : 128 * (co + 1)],
                                a2[:, ci, qsl],
                                start=(ci == 0), stop=(ci == NCI - 1),
                            )
                    for co in range(8):
                        hco = cog * 8 + co
                        nc.scalar.activation(
                            out=hsb[:, hco, qsl],
                            in_=pf[co],
                            func=Act.Gelu,
                            bias=b1t[:, hco : hco + 1],
                        )

        with nc.named_scope("ffn2"):
            wpool = wstream
            psp = ps_all
            for qb in range(cfg.NQB):
                qsl = slice(qb * QW, (qb + 1) * QW)
                py = [psp.tile([128, QW], f32, tag="mm", name=f"py{i}") for i in range(NCI)]
                for fi in range(NFF):
                    wt = wpool.tile([128, C], bf16, tag="w")
                    nc.sync.dma_start(out=wt, in_=w2[128 * fi : 128 * (fi + 1)])
                    for co in range(NCI):
                        nc.tensor.matmul(
                            py[co],
                            wt[:, 128 * co : 128 * (co + 1)],
                            hsb[:, fi, qsl],
                            start=(fi == 0), stop=(fi == NFF - 1),
                        )
                for co in range(NCI):
                    nc.vector.scalar_tensor_tensor(
                        out=yts[:, co, qsl],
                        in0=py[co],
                        scalar=b2t[:, co : co + 1],
                        in1=x2t[:, co, qsl],
                        op0=Alu.add,
                        op1=Alu.add,
                    )
        # per-co output DMAs so the store overlaps the ffn2 epilogue instead
        # of waiting for the whole yts tile
        ytr = yt.rearrange("(ci p) t -> ci p t", p=128)
        for co in range(NCI):
            nc.sync.dma_start(out=ytr[co], in_=yts[:, co, :])
        free_hsb()
        free_a2()
        free_yts()
        free_x2t()
        wstream.release()
        ps_all.release()
        free_prm()
        free_identsb()
        free_epst()
        free_oneshi()
        free_ones_row()
        free_ones128()
        free_onesf()

    nc.compile()
    return nc


def prep_core_inputs(cfg: Cfg, inputs: dict, b: int, j: int) -> dict:
    """Host-side slicing/permutation for core (batch b, parity j)."""
    T, TQ, NKB, MW = cfg.T, cfg.TQ, cfg.NKB, cfg.MW
    import ml_dtypes

    x = np.asarray(inputs["x"])
    perm = np.concatenate([np.arange(j, T, 2), np.arange(1 - j, T, 2)])
    xp = x[b][perm]  # [T, C]
    xpt = np.ascontiguousarray(xp.T).astype(ml_dtypes.bfloat16)

    qtok = perm[:TQ]
    ktok = perm
    mask = np.ones((NKB, 128, MW), dtype=np.float32)
    for kb in range(NKB):
        s = cfg.s_kb(kb)
        w = cfg.mask_w(kb)
        kt = ktok[128 * kb : 128 * (kb + 1)]  # [128]
        qt = qtok[s : s + w]  # [w]
        allowed = qt[None, :] >= kt[:, None]  # [128, w]
        mask[kb, :, :w] = np.where(allowed, 1.0, 0.0)
    return {"xpt": xpt, "msk": mask.astype(ml_dtypes.bfloat16)}


def prep_shared_inputs(cfg: Cfg, inputs: dict) -> dict:
    import ml_dtypes

    C = cfg.C
    f32 = np.float32
    bf16 = ml_dtypes.bfloat16

    def wq2d(w):  # [H, C, HD] -> [C, H*HD], pre-scaled for fp8e4m3
        w = np.asarray(w)
        return np.ascontiguousarray(
            w.transpose(1, 0, 2).reshape(C, C) * 64.0
        ).astype(ml_dtypes.float8_e4m3)

    HD = cfg.HD
    ident = np.zeros((HD, 256), dtype=bf16)
    ident[np.arange(HD), np.arange(HD)] = 1.0
    ident[np.arange(HD), 128 + HD + np.arange(HD)] = 1.0
    return {
        "ident": ident,
        "wq": wq2d(inputs["Wq"]),
        "wk": wq2d(inputs["Wk"]),
        "wv": wq2d(inputs["Wv"]),
        "wp": np.ascontiguousarray(np.asarray(inputs["Wproj"]) * 64.0).astype(
            ml_dtypes.float8_e4m3
        ),
        "w1": np.ascontiguousarray(inputs["W1"]).astype(bf16),
        "w2": np.ascontiguousarray(inputs["W2"]).astype(bf16),
        "prm": np.ascontiguousarray(
            np.concatenate(
                [
                    np.asarray(v, f32).reshape(-1, 128).T
                    for v in (
                        inputs["ln1_g"], inputs["ln1_b"],
                        inputs["ln2_g"], inputs["ln2_b"],
                        inputs["bproj"], inputs["b2"], inputs["b1"],
                    )
                ],
                axis=1,
            )
        ),
    }


def run(
    cfg: Cfg, inputs: dict, n_cores: int = 8, trace: bool = False, reps: int = 1
):
    from concourse.bass_utils import run_bass_kernel_spmd

    nc = build_nc(cfg, n_cores=n_cores)
    shared = prep_shared_inputs(cfg, inputs)
    in_maps = []
    cores = []
    for core in range(n_cores):
        b, j = divmod(core, 2)
        b = b % cfg.B
        in_maps.append({**prep_core_inputs(cfg, inputs, b, j), **shared})
        cores.append((b, j))
    res = None
    times = []
    for _ in range(max(1, reps)):
        r = run_bass_kernel_spmd(
            nc, in_maps, core_ids=list(range(n_cores)), trace=trace
        )
        if r.exec_time_ns is not None:
            times.append(r.exec_time_ns)
        if res is None or r.exec_time_ns is None or (
            res.exec_time_ns is not None and r.exec_time_ns < res.exec_time_ns
        ):
            res = r
    if times:
        print(f"exec times: {sorted(times)}")
    out = np.zeros((cfg.B, cfg.T, cfg.C), dtype=np.float32)
    for core, (b, j) in enumerate(cores):
        ytv = res.results[core]["yt"]  # [C, TQ]
        perm = np.concatenate(
            [np.arange(j, cfg.T, 2), np.arange(1 - j, cfg.T, 2)]
        )
        out[b, perm[: cfg.TQ], :] = ytv.T
    return out, res


def kernel(**inputs) -> np.ndarray:
    out, _ = run(Cfg(), inputs, n_cores=8, trace=False)
    return out


if __name__ == "__main__":
    # quick self-exercise at full size with random data
    rng = np.random.default_rng(0)
    cfg = Cfg()
    ins = {
        "x": rng.standard_normal((cfg.B, cfg.T, cfg.C)).astype(np.float32),
        "ln1_g": np.ones(cfg.C, np.float32),
        "ln1_b": np.zeros(cfg.C, np.float32),
        "ln2_g": np.ones(cfg.C, np.float32),
        "ln2_b": np.zeros(cfg.C, np.float32),
        "Wq": rng.standard_normal((cfg.H, cfg.C, cfg.HD)).astype(np.float32)
        * 0.02,
        "Wk": rng.standard_normal((cfg.H, cfg.C, cfg.HD)).astype(np.float32)
        * 0.02,
        "Wv": rng.standard_normal((cfg.H, cfg.C, cfg.HD)).astype(np.float32)
        * 0.02,
        "Wproj": rng.standard_normal((cfg.C, cfg.C)).astype(np.float32) * 0.02,
        "bproj": np.zeros(cfg.C, np.float32),
        "W1": rng.standard_normal((cfg.C, cfg.FF)).astype(np.float32) * 0.02,
        "b1": np.zeros(cfg.FF, np.float32),
        "W2": rng.standard_normal((cfg.FF, cfg.C)).astype(np.float32) * 0.02,
        "b2": np.zeros(cfg.C, np.float32),
    }
    y = kernel(**ins)
    print("ran, out", y.shape, y.dtype, float(np.abs(y).max()))

